# revision 16
# baseline (speedup 1.0000x reference)
"""Single-head attention (B=4, S=2048, E=1024) on 8 TRN2 NeuronCores.

Sharding: core c -> (batch b = c//2, sequence-half h = c%2).

Core c -> (batch b = c//2, half h = c%2). Each core computes K^T and V
only for its OWN 1024-column half of the sequence, then the pair
all-gathers both (through DRAM bounce buffers) so each core assembles
the full K^T [1024f, 2048k] and V [2048k, 1024f] in ABSOLUTE key order
(gather block hh is rank hh's half -> no per-core indexing anywhere;
the SPMD program is identical across cores).

Saves 256 of 1184 matmuls per core vs the duplicate-K/V version and
cuts input DMA from 14MB to 8MB (x ships as just the core's own half).
"""

import numpy as np
import ml_dtypes

import concourse.bass as bass
import concourse.tile as tile
from concourse import bacc, mybir
from concourse.bass_utils import run_bass_kernel_spmd

B, S, E = 4, 2048, 1024
N_CORES = 8
SQ = S // 2
P = 128
NT = 512
ET = E // P        # 8
KT = S // P        # 16
KTH = SQ // P      # 8 own-half k tiles
FP32 = mybir.dt.float32
BF16 = mybir.dt.bfloat16
SCALE = 1.0 / np.sqrt(E).astype(np.float32)
SHIFT = -4.0
PAIRS = [[0, 1], [2, 3], [4, 5], [6, 7]]


def build_kernel(ctx, tc, io):
    nc = tc.nc
    xo, wqT, wkT, wvT, bq, bk, bv, outT = (
        io["xo"], io["wqT"], io["wkT"], io["wvT"],
        io["bq"], io["bk"], io["bv"], io["outT"],
    )

    singles = ctx.enter_context(tc.tile_pool(name="singles", bufs=1))
    results = ctx.enter_context(tc.tile_pool(name="results", bufs=1))
    # kh/vh halves are dead once bounced out to DRAM; p_sb reuses the slot.
    xp_pool = ctx.enter_context(tc.tile_pool(name="xp", bufs=1))
    outp = ctx.enter_context(tc.tile_pool(name="outp", bufs=3))
    dram = ctx.enter_context(tc.tile_pool(name="dram", bufs=1, space="DRAM"))
    ps_main = ctx.enter_context(tc.tile_pool(name="ps_main", bufs=4, space="PSUM"))
    ps_sums = ctx.enter_context(tc.tile_pool(name="ps_sums", bufs=2, space="PSUM"))
    ps_out = ctx.enter_context(tc.tile_pool(name="ps_out", bufs=2, space="PSUM"))

    # ---- ScalarE LUT warm-up: force the Identity/Exp ACT_TABLE_LOADs to
    # happen before the input DMA stream, not queued behind it (a late
    # table load stalls every ACTIVATE -> PSUM recycle -> TensorE).
    warm = singles.tile([1, 4], FP32)
    warmb = singles.tile([1, 1], FP32)
    nc.vector.memset(warm, 0.0)
    nc.vector.memset(warmb, 0.0)
    nc.scalar.activation(out=warm[:, 0:2], in_=warm[:, 0:2],
                         func=mybir.ActivationFunctionType.Identity,
                         bias=warmb, scale=1.0)
    nc.scalar.activation(out=warm[:, 2:4], in_=warm[:, 2:4],
                         func=mybir.ActivationFunctionType.Exp,
                         bias=warmb, scale=1.0)

    # ---- input staging, first-use order. Tiny bias DMAs go FIRST (they
    # gate the first ACTIVATEs; queued behind the slabs they land ~35us in).
    bq_sb = singles.tile([P, ET], FP32)
    bk_sb = singles.tile([P, ET], FP32)
    bv_bc = singles.tile([P, E], FP32)
    nc.gpsimd.dma_start(out=bk_sb, in_=bk.rearrange("(t p) -> p t", p=P))
    nc.gpsimd.dma_start(out=bq_sb, in_=bq.rearrange("(t p) -> p t", p=P))
    nc.gpsimd.dma_start(out=bv_bc, in_=bv.partition_broadcast(P))

    # Slab DMAs alternate between sync and vector queues: issue costs
    # ~0.6us per dma_start per engine, so single-engine issue of 16 slabs
    # would serialize ~10us against an 11us transfer floor.
    wk_sb = singles.tile([P, ET, E], BF16)
    wv_sb = singles.tile([P, ET, E], BF16)
    wq_sb = singles.tile([P, ET, E], BF16)
    xo_sb = singles.tile([P, ET, SQ], BF16)
    for t in range(ET):
        r = slice(t * P, (t + 1) * P)
        nc.sync.dma_start(out=wk_sb[:, t, :], in_=wkT[r, :])
        nc.sync.dma_start(out=xo_sb[:, t, :], in_=xo[r, :])
    for t in range(ET):
        r = slice(t * P, (t + 1) * P)
        nc.sync.dma_start(out=wv_sb[:, t, :], in_=wvT[r, :])
    for t in range(ET):
        r = slice(t * P, (t + 1) * P)
        nc.sync.dma_start(out=wq_sb[:, t, :], in_=wqT[r, :])

    ones_sb = singles.tile([P, 1], BF16)
    nc.vector.memset(ones_sb, 1.0)
    shift_sb = singles.tile([P, 1], FP32)
    nc.vector.memset(shift_sb, SHIFT)

    qT_sb = results.tile([P, ET, SQ], BF16)
    kT_sb = results.tile([P, ET, S], BF16)
    v_sb = results.tile([P, KT, E], BF16)
    scr = xp_pool.tile([P, KT, SQ], BF16, tag="xp")  # kh: [:, 0:8, :], vh: [:, 8:16, :]
    kh_sb = scr[:, 0:ET, :]
    vh_sb = scr[:, ET:KT, :]

    # K gather is split into two k'-halves so the collective firmware
    # latency pipelines against the remaining projections; V is one gather
    # (only needed by PV, much later).
    bounce_k = [dram.tile([SQ, NT], BF16, name=f"bounce_k{i}", tag=f"bk{i}")
                for i in range(2)]
    gath_k = [dram.tile([S, NT], BF16, name=f"gath_k{i}", tag=f"gk{i}")
              for i in range(2)]
    bounce_v = dram.tile([SQ, E], BF16)
    gath_v = dram.tile([S, E], BF16)

    ident = mybir.ActivationFunctionType.Identity

    # ---- K^T own half [f, k'], one k'-half at a time: compute -> bounce
    # -> gather -> back-DMA (absolute k order in kT_sb)
    for ks in range(SQ // NT):
        kr = slice(ks * NT, (ks + 1) * NT)
        for ft in range(ET):
            fr = slice(ft * P, (ft + 1) * P)
            ps = ps_main.tile([P, NT], FP32)
            for et in range(ET):
                nc.tensor.matmul(ps, lhsT=wk_sb[:, et, fr], rhs=xo_sb[:, et, kr],
                                 start=(et == 0), stop=(et == ET - 1))
            nc.scalar.activation(out=kh_sb[:, ft, kr], in_=ps, func=ident,
                                 bias=bk_sb[:, ft:ft + 1], scale=1.0)
        for t in range(ET):
            nc.sync.dma_start(out=bounce_k[ks][t * P:(t + 1) * P, :],
                              in_=kh_sb[:, t, kr])
        nc.gpsimd.collective_compute(
            "AllGather", mybir.AluOpType.bypass, replica_groups=PAIRS,
            ins=[bounce_k[ks][:, :]], outs=[gath_k[ks][:, :]])
        for hh in range(2):
            for t in range(ET):
                nc.sync.dma_start(
                    out=kT_sb[:, t, hh * SQ + ks * NT: hh * SQ + (ks + 1) * NT],
                    in_=gath_k[ks][hh * SQ + t * P: hh * SQ + (t + 1) * P, :])

    # ---- V own half  [k', f] -> bounce -> gather -> back
    for kt in range(KTH):
        kr = slice(kt * P, (kt + 1) * P)
        for fs in range(E // NT):
            fr = slice(fs * NT, (fs + 1) * NT)
            ps = ps_main.tile([P, NT], FP32)
            for et in range(ET):
                nc.tensor.matmul(ps, lhsT=xo_sb[:, et, kr], rhs=wv_sb[:, et, fr],
                                 start=(et == 0), stop=(et == ET - 1))
            nc.vector.tensor_add(vh_sb[:, kt, fr], ps, bv_bc[:, fr])
        nc.sync.dma_start(out=bounce_v[kt * P:(kt + 1) * P, :], in_=vh_sb[:, kt, :])
    nc.gpsimd.collective_compute(
        "AllGather", mybir.AluOpType.bypass, replica_groups=PAIRS,
        ins=[bounce_v[:, :]], outs=[gath_v[:, :]])
    for kt in range(KT):
        nc.sync.dma_start(out=v_sb[:, kt, :], in_=gath_v[kt * P:(kt + 1) * P, :])

    # ---- Q^T = Wq x_own + bq   [f, q]  (overlaps the gathers)
    # qs-outer: all of qs=0 lands first so scores(qs=0) isn't gated on the
    # last ACT of the whole Q pass.
    for qs in range(SQ // NT):
        qr = slice(qs * NT, (qs + 1) * NT)
        for ft in range(ET):
            fr = slice(ft * P, (ft + 1) * P)
            ps = ps_main.tile([P, NT], FP32)
            for et in range(ET):
                nc.tensor.matmul(ps, lhsT=wq_sb[:, et, fr], rhs=xo_sb[:, et, qr],
                                 start=(et == 0), stop=(et == ET - 1))
            nc.scalar.activation(out=qT_sb[:, ft, qr], in_=ps, func=ident,
                                 bias=bq_sb[:, ft:ft + 1], scale=1.0)

    # ---- scores^T and P = exp(S^T * scale + shift)   [k, q]
    # qs-outer: finish all kt for one q-slice, then its denominators,
    # so the sums/PV phases aren't gated on the very last exp of both
    # q-slices.
    p_sb = xp_pool.tile([P, KT, SQ], BF16, tag="xp")
    recip_sb = singles.tile([1, SQ], FP32)
    for qs in range(SQ // NT):
        qr = slice(qs * NT, (qs + 1) * NT)
        for kt in range(KT):
            kr = slice(kt * P, (kt + 1) * P)
            ps = ps_main.tile([P, NT], FP32)
            for et in range(ET):
                nc.tensor.matmul(ps, lhsT=kT_sb[:, et, kr], rhs=qT_sb[:, et, qr],
                                 start=(et == 0), stop=(et == ET - 1))
            nc.scalar.activation(out=p_sb[:, kt, qr], in_=ps,
                                 func=mybir.ActivationFunctionType.Exp,
                                 bias=shift_sb[:, 0:1], scale=float(SCALE))
        ps_d = ps_sums.tile([1, NT], FP32)
        for kt in range(KT):
            nc.tensor.matmul(ps_d, lhsT=ones_sb, rhs=p_sb[:, kt, qr],
                             start=(kt == 0), stop=(kt == KT - 1))
        nc.vector.reciprocal(out=recip_sb[:, qr], in_=ps_d)
    recip_dram = dram.tile([1, SQ], FP32)
    nc.sync.dma_start(out=recip_dram, in_=recip_sb)
    recip_bc = singles.tile([P, SQ], FP32)
    nc.sync.dma_start(out=recip_bc, in_=recip_dram[0, :].partition_broadcast(P))

    # ---- O^T = V^T P, normalize, out
    for ft in range(ET):
        fr = slice(ft * P, (ft + 1) * P)
        for qs in range(SQ // NT):
            qr = slice(qs * NT, (qs + 1) * NT)
            ps = ps_out.tile([P, NT], FP32)
            for kt in range(KT):
                nc.tensor.matmul(ps, lhsT=v_sb[:, kt, fr], rhs=p_sb[:, kt, qr],
                                 start=(kt == 0), stop=(kt == KT - 1))
            ot = outp.tile([P, NT], FP32)
            nc.vector.tensor_mul(ot, ps, recip_bc[:, qr])
            nc.sync.dma_start(out=outT[fr, qr], in_=ot)


def build_program():
    nc = bacc.Bacc("TRN2", target_bir_lowering=False, debug=False,
                   num_devices=N_CORES)
    io = {
        "xo": nc.dram_tensor("xo", [E, SQ], BF16, kind="ExternalInput").ap(),
        "wqT": nc.dram_tensor("wqT", [E, E], BF16, kind="ExternalInput").ap(),
        "wkT": nc.dram_tensor("wkT", [E, E], BF16, kind="ExternalInput").ap(),
        "wvT": nc.dram_tensor("wvT", [E, E], BF16, kind="ExternalInput").ap(),
        "bq": nc.dram_tensor("bq", [E], FP32, kind="ExternalInput").ap(),
        "bk": nc.dram_tensor("bk", [E], FP32, kind="ExternalInput").ap(),
        "bv": nc.dram_tensor("bv", [E], FP32, kind="ExternalInput").ap(),
        "outT": nc.dram_tensor("outT", [E, SQ], FP32, kind="ExternalOutput").ap(),
    }
    from contextlib import ExitStack
    with tile.TileContext(nc) as tc:
        with ExitStack() as ctx:
            build_kernel(ctx, tc, io)
    nc.compile()
    return nc


def make_in_maps(x, wq_w, wq_b, wk_w, wk_b, wv_w, wv_b):
    bf = ml_dtypes.bfloat16
    xT_all = np.ascontiguousarray(np.transpose(np.asarray(x, np.float32),
                                               (0, 2, 1))).astype(bf)
    wqT = np.ascontiguousarray(np.asarray(wq_w, np.float32).T).astype(bf)
    wkT = np.ascontiguousarray(np.asarray(wk_w, np.float32).T).astype(bf)
    wvT = np.ascontiguousarray(np.asarray(wv_w, np.float32).T).astype(bf)
    bq = np.asarray(wq_b, np.float32)
    bk = np.asarray(wk_b, np.float32)
    bv = np.asarray(wv_b, np.float32)
    in_maps = []
    for c in range(N_CORES):
        b, h = divmod(c, 2)
        in_maps.append({
            "xo": np.ascontiguousarray(xT_all[b][:, h * SQ:(h + 1) * SQ]),
            "wqT": wqT, "wkT": wkT, "wvT": wvT,
            "bq": bq, "bk": bk, "bv": bv,
        })
    return in_maps


def assemble_out(results):
    out = np.empty((B, S, E), np.float32)
    for c in range(N_CORES):
        b, h = divmod(c, 2)
        out[b, h * SQ:(h + 1) * SQ, :] = results[c]["outT"].T
    return out


_NC_CACHE = None


def kernel(x, wq_w, wq_b, wk_w, wk_b, wv_w, wv_b):
    global _NC_CACHE
    if _NC_CACHE is None:
        _NC_CACHE = build_program()
    in_maps = make_in_maps(x, wq_w, wq_b, wk_w, wk_b, wv_w, wv_b)
    try:
        res = run_bass_kernel_spmd(_NC_CACHE, in_maps, list(range(N_CORES)))
    except Exception:
        # transient axon/device hiccups happen; one retry
        res = run_bass_kernel_spmd(_NC_CACHE, in_maps, list(range(N_CORES)))
    return assemble_out(res.results)
